# revision 20
# baseline (speedup 1.0000x reference)
"""HaarMSELoss kernel for Trainium2 (8 NeuronCores, data-parallel).

Math: the 2x2 Haar transform used by the reference is (up to the 0.5
scaling) an orthogonal Hadamard transform, so for each 2x2 block
LL^2+LH^2+HL^2+HH^2 == a^2+b^2+c^2+d^2 of the block entries of
(input - target).  Hence

  loss = sum_bands mean((haar(x)-haar(y))^2)
       = sum((x-y)^2) / (B*C*(H/2)*(W/2))

i.e. a pure squared-difference reduction.  Each core reduces 1/8 of the
elements; the host sums the 8x128 per-partition partials (f64) and
divides.

Layout: per core the two chunks are interleaved host-side at TILE
granularity into one [128, 2*FREE] array: tile t occupies the
contiguous span [2*off_t, 2*off_t + 2*w_t) of each partition row, x
half then y half.  That makes each tile's DRAM run contiguous per
partition, so balance_dma_aps merges it into a single 2*w-element
last dim and the DGE emits 32 KB descriptors (vs 16 KB with the
[128, 2, FREE] layout, where the x/y halves sit 128 KB apart) —
halving descriptor/packet count per SDMA engine.

Streaming: per-core DMA bandwidth caps at ~400-420 GB/s (HBM domain
share) and only full-128-partition dma_starts reach it (partial
partition ranges engage one mux half per SBUF AXI port and run at half
rate), so every load is a full-width [128, 2, w] tile on the single
Sync HWDGE queue.  Tile widths taper at the end (2048, 1024, 512, 512)
so the serial sub+square tail after the last byte lands is small.

Correctness: a dma_start's then_inc(sem, 16) is delivered as 16
independent +1 increments (one per SDMA engine as it finishes its
slice), so a cumulative wait sem >= 16*(t+1) can pass spuriously when
engines skew across tiles — reading a tile before it fully landed.
Each tile therefore gets its OWN semaphore and consumers wait for
exactly 16 on it; this is exact under any engine skew.

Raw bass pipeline (explicit sems; every wait is a standalone
single-sem wait):
  SP  : dma loads (slot-recycled against ACT), final stats store
  DVE : d = x - y in place
  ACT : stats[:,t] = sum(d^2) via activation(Square, accum_out)
"""

import numpy as np

_B, _C, _H, _W = 4, 32, 512, 512
_TOTAL = _B * _C * _H * _W          # 33_554_432
_NCORES = 8
_PER_CORE = _TOTAL // _NCORES       # 4_194_304
_P = 128
_FREE = _PER_CORE // _P             # 32_768 f32 per partition per tensor
_WIDTHS = ([4096] * 5 + [2048] * 3
           + [1536, 1024, 1024, 768, 768, 512, 256, 256])  # sums to 32768
_NT = len(_WIDTHS)                  # 16 tiles
_OFFS = [sum(_WIDTHS[:i]) for i in range(_NT)]
_NBUF = 5
_WMAX = 4096
_DIVISOR = float(_TOTAL // 4)       # 8_388_608  (elements per subband)

_CACHE = {}


def _build_nc():
    from contextlib import ExitStack
    import concourse.bass as bass
    import concourse.mybir as mybir

    f32 = mybir.dt.float32
    nc = bass.Bass("TRN2", target_bir_lowering=False)
    xy = nc.dram_tensor("xy", [_P, 2 * _FREE], f32, kind="ExternalInput")
    out = nc.dram_tensor("out", [_P, _NT], f32, kind="ExternalOutput")

    ctx = ExitStack()
    nc._ctx = ctx  # keep SBUF/semaphore handles alive for compile
    slots = [ctx.enter_context(nc.sbuf_tensor(f"slot{i}", [_P, 2 * _WMAX], f32))
             for i in range(_NBUF)]
    stats = ctx.enter_context(nc.sbuf_tensor([_P, _NT], f32))
    zbias = ctx.enter_context(nc.sbuf_tensor([_P, 1], f32))
    tile_sems = [ctx.enter_context(nc.semaphore(f"tile_sem{t}"))
                 for t in range(_NT)]
    store_sem = ctx.enter_context(nc.semaphore("store_sem"))
    dve_sem = ctx.enter_context(nc.semaphore("dve_sem"))
    act_sem = ctx.enter_context(nc.semaphore("act_sem"))
    block = ctx.enter_context(nc.Block())

    @block.sync
    def _(sync):
        for t in range(_NT):
            if t >= _NBUF:
                # slot free once ACT (last reader) finished tile t-NBUF
                sync.wait_ge(act_sem, t - _NBUF + 1)
            w, o = _WIDTHS[t], _OFFS[t]
            st = slots[t % _NBUF]
            sync.dma_start(
                out=st[:, :2 * w], in_=xy[:, 2 * o:2 * (o + w)]
            ).then_inc(tile_sems[t], 16)
        # act_sem increments fire on ACTIVATION_READ_ACCUMULATOR complete,
        # so stats columns are final once their square's RA retired.  Store
        # the first NT-1 columns overlapped with the last square; only the
        # final 512 B column store sits on the critical path.
        sync.wait_ge(act_sem, _NT - 1)
        sync.dma_start(
            out=out[:, :_NT - 1], in_=stats[:, :_NT - 1]
        ).then_inc(store_sem, 16)
        sync.wait_ge(act_sem, _NT)
        with nc.allow_non_contiguous_dma(reason="512B final-column store"):
            sync.dma_start(
                out=out[:, _NT - 1:], in_=stats[:, _NT - 1:]
            ).then_inc(store_sem, 16)
        sync.wait_ge(store_sem, 32)  # both stores landed

    @block.vector
    def _(vector):
        vector.memset(zbias[:], 0.0).then_inc(dve_sem, 1)
        for t in range(_NT):
            w = _WIDTHS[t]
            st = slots[t % _NBUF]
            vector.wait_ge(tile_sems[t], 16)
            vector.tensor_sub(st[:, :w], st[:, :w], st[:, w:2 * w]) \
                  .then_inc(dve_sem, 1)

    @block.scalar
    def _(scalar):
        for t in range(_NT):
            w = _WIDTHS[t]
            st = slots[t % _NBUF]
            scalar.wait_ge(dve_sem, t + 2)
            scalar.activation(
                st[:, :w], st[:, :w],
                mybir.ActivationFunctionType.Square,
                bias=zbias[:, 0:1], accum_out=stats[:, t:t + 1],
            ).then_inc(act_sem, 1)

    ctx.close()
    return nc


def _run(in_maps, trace=False):
    from concourse.bass_utils import run_bass_kernel_spmd

    if "nc" not in _CACHE:
        _CACHE["nc"] = _build_nc()
    return run_bass_kernel_spmd(
        _CACHE["nc"], in_maps, list(range(_NCORES)), trace=trace
    )


def _make_in_maps(input, target):
    xs = np.asarray(input, dtype=np.float32).reshape(_NCORES, _P, _FREE)
    ys = np.asarray(target, dtype=np.float32).reshape(_NCORES, _P, _FREE)
    maps = []
    for c in range(_NCORES):
        xy = np.empty((_P, 2 * _FREE), dtype=np.float32)
        for t in range(_NT):
            w, o = _WIDTHS[t], _OFFS[t]
            xy[:, 2 * o:2 * o + w] = xs[c][:, o:o + w]
            xy[:, 2 * o + w:2 * (o + w)] = ys[c][:, o:o + w]
        maps.append({"xy": xy})
    return maps


def _finish(results):
    total = 0.0
    for r in results:
        total += r["out"].astype(np.float64).sum()
    return np.array(total / _DIVISOR, dtype=np.float32)


def kernel(input, target):
    res = _run(_make_in_maps(input, target), trace=False)
    return _finish(res.results)
